# revision 35
# baseline (speedup 1.0000x reference)
"""CMADE ensemble kernel for 8 TRN2 NeuronCores.

Problem: B=16 binary-masked 4-layer MLPs (96 -> 1024 -> 1024 -> 1024 -> 64)
over the same N=4096 batch; output = mean over the 16 masks.

Strategy: data-parallel over the batch N -- each core takes 512 rows and runs
all 16 masked MLPs on them, accumulating the final-layer outputs of all 16
masks into a single PSUM tile; no inter-core collective is needed.

v2 design (vs the mask-streaming baseline):
- Masked weights (W.T * M) are precomputed on the HOST and streamed to the
  device as ready-to-matmul fp16/fp8 tiles. This removes the entire
  DVE mask-multiply pipeline (340us of vector-engine work) and the
  DMA->DVE->PE dependency chain that stalled the PE at startup and between
  early masks. The device is pure weight-streaming + matmul + drains.
- Non-fp8 data is fp16 (not bf16): same PE speed, 8x less quantization
  error (rel err 3.9e-4 vs 3.3e-3), buying error budget for fp8.
- Layer 2's first 4 k-slices (half of L2 = 25% of the mid-layer MACs) run
  as fp8e4 DoubleRow pair-matmuls (2 fp8 MACs/cycle): weights scaled x64,
  activations x32, both quantized e4m3; psum stays exact fp32. Simulated
  end-to-end rel err 1.63e-2 (gate 2e-2). The scales are folded into host
  weight/bias prep so every drain is a plain max(psum+bias,0).
- Weight streams ride 3 independent DMA queues (sync-HWDGE: mw1,
  scalar-HWDGE: mw2, gpsimd-SWDGE: mw0/mw3), one coarse DMA per
  mask-layer (per-DMA descriptor-gen is ~1.2us, so few big DMAs beat many
  k-sliced ones); mask 0/1 are chunked so the first consumers start early.
- Finalize: the two L3 column-group psum halves are summed with a tiny
  fp32 selector matmul and the [64, 512] result is DMAed out transposed;
  the host does the final [512, 64] transpose (it concatenates anyway).
"""

import numpy as np
import ml_dtypes

from concourse import bacc
import concourse.bass as bass
import concourse.mybir as mybir
import concourse.tile as tile
from concourse.bass_utils import run_bass_kernel_spmd

F16 = np.float16
E4 = ml_dtypes.float8_e4m3

N = 4096
B = 16
NCORES = 8
NLOC = N // NCORES           # 512 batch rows per core
D_IN = 96
H = 1024
D_OUT = 64
KT = H // 128                # 8 k-tiles for the 1024-wide dims
NF8 = 4                      # leading k-slices of L2 in fp8 (DoubleRow pairs)
NF81 = 2                     # leading k-slices of L1 in fp8 (DoubleRow pairs)
SW = 64.0                    # L2 fp8 weight scale (folded into host weights)
SA = 32.0                    # a1 storage scale (folded into L1 wts/bias)
SA0 = 8.0                    # a0 fp8 storage scale (folded into mw0 cols 0:256)
SW1 = 4.0                    # L1 fp8 weight scale (SA0*SW1 == SA)

TRACE = False
LAST_RESULT = None

_CACHE = {}


def _ensure_ntff_hook():
    """The agent image's antenv lacks axon_hooks; reconstruct the NTFF
    profile hook from trn_agent_boot so trace=True yields exec_time_ns."""
    import sys as _sys
    import types
    try:
        from antenv import axon_hooks  # noqa: F401
        return
    except ImportError:
        pass
    import antenv
    import concourse.bass_utils as _bu
    _bu.upload_artifacts = lambda tmpdir: tmpdir  # zero-egress container
    holder = {}
    mod = types.ModuleType("antenv.axon_hooks")
    mod.set_axon_ntff_profile_hook = lambda h: holder.__setitem__("h", h)
    mod.get_axon_ntff_profile_hook = lambda: holder.get("h")
    _sys.modules["antenv.axon_hooks"] = mod
    antenv.axon_hooks = mod
    from trn_agent_boot.trn_boot import _ntff_profile_via_ctypes
    mod.set_axon_ntff_profile_hook(
        _ntff_profile_via_ctypes("/opt/axon/libaxon_pjrt.so"))


def _build_graph():
    f32 = mybir.dt.float32
    f16 = mybir.dt.float16
    f8 = mybir.dt.float8e4
    nc = bacc.Bacc("TRN2", target_bir_lowering=False, debug=False,
                   num_devices=NCORES)

    # ---- I/O ----
    xyT_d = nc.dram_tensor("xyT", [D_IN, NLOC], f16, kind="ExternalInput")
    mw0_d = nc.dram_tensor("mw0", [B, D_IN, H], f16, kind="ExternalInput")
    mw1f_d = nc.dram_tensor("mw1f", [B, 128, NF81, H], f8,
                            kind="ExternalInput")
    mw1_d = nc.dram_tensor("mw1", [B, 128, KT - NF81, H], f16,
                           kind="ExternalInput")
    mw2f_d = nc.dram_tensor("mw2f", [B, 128, NF8, H], f8, kind="ExternalInput")
    mw2h_d = nc.dram_tensor("mw2h", [B, 128, KT - NF8, H], f16,
                            kind="ExternalInput")
    mw3_d = nc.dram_tensor("mw3", [B, 128, KT, D_OUT], f16,
                           kind="ExternalInput")
    bias_d = nc.dram_tensor("biasp", [128, 3 * KT], f32, kind="ExternalInput")
    b3_d = nc.dram_tensor("b3r", [D_OUT, 1], f32, kind="ExternalInput")
    sel_d = nc.dram_tensor("sel", [128, D_OUT], f16, kind="ExternalInput")
    out_d = nc.dram_tensor("out", [D_OUT, NLOC], f32, kind="ExternalOutput")

    relu = mybir.ActivationFunctionType.Relu
    iden = mybir.ActivationFunctionType.Identity
    add_op = mybir.AluOpType.add
    max_op = mybir.AluOpType.max
    DR = mybir.MatmulPerfMode.DoubleRow

    from contextlib import ExitStack
    with tile.TileContext(nc) as tc, ExitStack() as ctx:
        const = ctx.enter_context(tc.tile_pool(name="const", bufs=1))
        mw0p = ctx.enter_context(tc.tile_pool(name="mw0", bufs=2))
        mw1fp = ctx.enter_context(tc.tile_pool(name="mw1f", bufs=2))
        mw1p = ctx.enter_context(tc.tile_pool(name="mw1", bufs=2))
        mw2fp = ctx.enter_context(tc.tile_pool(name="mw2f", bufs=2))
        mw2hp = ctx.enter_context(tc.tile_pool(name="mw2h", bufs=2))
        mw3p = ctx.enter_context(tc.tile_pool(name="mw3", bufs=2))
        apool = ctx.enter_context(tc.tile_pool(name="act", bufs=2))
        pspool = ctx.enter_context(tc.tile_pool(name="ps", bufs=7, space="PSUM"))
        ps3pool = ctx.enter_context(tc.tile_pool(name="ps3", bufs=1, space="PSUM"))
        finp = ctx.enter_context(tc.tile_pool(name="fin", bufs=2))

        # ---- startup-critical loads: mw0[0] rides the scalar HWDGE queue
        # (it ramps to full rate immediately); xyT + biases lead the sync
        # queue ahead of the mw1 chunks
        xyT = const.tile([D_IN, NLOC], f16, tag="xyT")
        nc.sync.dma_start(xyT[:], xyT_d[:])
        bt = const.tile([128, 3 * KT], f32, tag="bt")
        nc.sync.dma_start(bt[:], bias_d[:])

        # ---- per-mask weight streams; mask 0/1 chunked so early k-slices
        # post their completion semaphores before the whole layer arrives
        def fetch(b):
            mw0t = mw0p.tile([D_IN, H], f16, tag="mw0", bufs=3,
                             name=f"mw0_{b}")
            if b == 0:
                nc.scalar.dma_start(mw0t[:], mw0_d[b])
            else:
                nc.gpsimd.dma_start(mw0t[:], mw0_d[b])
            mw1ft = mw1fp.tile([128, NF81, H], f8, tag="mw1f", bufs=3,
                               name=f"mw1f_{b}")
            mw1t = mw1p.tile([128, KT - NF81, H], f16, tag="mw1", bufs=3,
                             name=f"mw1_{b}")
            if b == 0:
                # mask 0's first chunks on sync (behind xyT+bias), the rest
                # on the otherwise-idle scalar queue -- its few issues
                # retire before the first drains need the scalar engine
                nc.sync.dma_start(mw1ft[:], mw1f_d[b])
                nc.sync.dma_start(mw1t[:, 0:2, :], mw1_d[b][:, 0:2, :])
                nc.scalar.dma_start(mw1t[:, 2:4, :], mw1_d[b][:, 2:4, :])
                nc.scalar.dma_start(mw1t[:, 4:6, :], mw1_d[b][:, 4:6, :])
            elif b == 1:
                # split mask 1 across the scalar and sync queues so its L1
                # weights land before the cold-phase k-loop reaches them
                nc.scalar.dma_start(mw1ft[:], mw1f_d[b])
                nc.scalar.dma_start(mw1t[:, 0:2, :], mw1_d[b][:, 0:2, :])
                nc.sync.dma_start(mw1t[:, 2:4, :], mw1_d[b][:, 2:4, :])
                nc.sync.dma_start(mw1t[:, 4:6, :], mw1_d[b][:, 4:6, :])
            else:
                nc.sync.dma_start(mw1ft[:], mw1f_d[b])
                nc.sync.dma_start(mw1t[:], mw1_d[b])
            # mw2/mw3 ride the gpsimd SWDGE queue: DMA-issue instructions can
            # block on semaphore-reuse waits, and gpsimd has no compute
            # duties to stall (the scalar engine must stay free for drains)
            mw2ft = mw2fp.tile([128, NF8, H], f8, tag="mw2f", bufs=3,
                               name=f"mw2f_{b}")
            mw2ht = mw2hp.tile([128, KT - NF8, H], f16, tag="mw2h", bufs=3,
                               name=f"mw2h_{b}")
            if b < 2:
                nc.gpsimd.dma_start(mw2ft[:, 0:2, :], mw2f_d[b][:, 0:2, :])
                nc.gpsimd.dma_start(mw2ht[:, 0:2, :], mw2h_d[b][:, 0:2, :])
                nc.gpsimd.dma_start(mw2ft[:, 2:4, :], mw2f_d[b][:, 2:4, :])
                nc.gpsimd.dma_start(mw2ht[:, 2:4, :], mw2h_d[b][:, 2:4, :])
            else:
                nc.gpsimd.dma_start(mw2ft[:], mw2f_d[b])
                nc.gpsimd.dma_start(mw2ht[:], mw2h_d[b])
            mw3t = mw3p.tile([128, KT, D_OUT], f16, tag="mw3", bufs=3,
                             name=f"mw3_{b}")
            nc.gpsimd.dma_start(mw3t[:], mw3_d[b])
            return mw0t, mw1ft, mw1t, mw2ft, mw2ht, mw3t



        # psum -> sbuf drain: all scales are folded into weights/biases, so
        # every drain is max(psum + bias, 0), alternating Scalar/Vector
        def drain(at, ps, col, dve):
            if dve:
                nc.vector.tensor_scalar(at, ps, bt[:, col:col + 1], 0.0,
                                        add_op, max_op)
            else:
                nc.scalar.activation(at, ps, relu, bias=bt[:, col:col + 1])

        # latency-critical drains (a0 feeds L1's k-loop almost immediately):
        # split each tile across both engines by column half -- engine time
        # scales with the free dim, so halving columns halves the latency
        def drain_split(at, ps, col):
            h = NLOC // 2
            nc.scalar.activation(at[:, 0:h], ps[:, 0:h], relu,
                                 bias=bt[:, col:col + 1])
            nc.vector.tensor_scalar(at[:, h:NLOC], ps[:, h:NLOC],
                                    bt[:, col:col + 1], 0.0,
                                    add_op, max_op)

        ps3 = ps3pool.tile([128, NLOC], f32, tag="ps3")

        def layer3_pairs(b, mw3t, a2, kps):
            # M=64 fills half the PE columns; pair k-tiles into concurrent
            # col-group matmuls writing disjoint psum partition halves
            for kp in kps:
                k0, k1 = 2 * kp, 2 * kp + 1
                st = (b == 0 and kp == 0)
                sp = (b == B - 1 and kp == KT // 2 - 1)
                nc.tensor.matmul(ps3[0:D_OUT, :], mw3t[:, k0, :], a2[k0][:],
                                 start=st, stop=sp, tile_position=(0, 0))
                nc.tensor.matmul(ps3[D_OUT:128, :], mw3t[:, k1, :], a2[k1][:],
                                 start=st, stop=sp, tile_position=(0, 64))

        fetched = {0: fetch(0)}
        b3t = const.tile([D_OUT, 1], f32, tag="b3t")
        nc.gpsimd.dma_start(b3t[:], b3_d[:])
        selt = const.tile([128, D_OUT], f16, tag="selt")
        nc.gpsimd.dma_start(selt[:], sel_d[:])
        fetched[1] = fetch(1)

        # ---- layer 0: [96] -> [1024]; m-tiles 0..1 drain to the fp8
        # pair tile feeding L1's DoubleRow slices (psum is 8*z0 there --
        # the x8 is folded into mw0's first 256 columns)
        def layer0(b, mw0t):
            a0f = apool.tile([128, 2, NLOC], f8, tag="a0f",
                             name=f"a0f_{b}")
            a0 = [None] * KT
            for m in range(KT):
                ps = pspool.tile([128, NLOC], f32, tag="ps",
                                 name=f"ps_a0_{b}_{m}")
                nc.tensor.matmul(ps[:], mw0t[:, m * 128:(m + 1) * 128],
                                 xyT[:], start=True, stop=True)
                if m < NF81:
                    drain_split(a0f[:, m, :], ps[:], m)
                else:
                    at = apool.tile([128, NLOC], f16, tag=f"a0_{m}",
                                    name=f"a0_{b}_{m}")
                    drain_split(at[:], ps[:], m)
                    a0[m] = at
            return a0f, a0

        prev = None  # (b, mw3t, a2) pending layer-3
        a0_cache = {}
        for b in range(B):
            if b + 2 < B:
                fetched[b + 2] = fetch(b + 2)
            mw0t, mw1ft, mw1t, mw2ft, mw2ht, mw3t = fetched.pop(b)
            if b in a0_cache:
                a0f, a0 = a0_cache.pop(b)
            else:
                a0f, a0 = layer0(b, mw0t)

            if prev is not None and b != 1:
                layer3_pairs(*prev, range(KT // 2))

            # ---- layer 1: k-outer over m-halves; m 0-3 drain to fp8 pair
            # tiles (L2's DoubleRow inputs), m 4-7 to fp16
            a1f = [apool.tile([128, 2, NLOC], f8, tag=f"a1f_{p}",
                              name=f"a1f_{b}_{p}") for p in range(NF8 // 2)]
            a1h = [None] * (KT - NF8)
            for half in range(2):
                ms = range(half * 4, half * 4 + 4)
                pss = [pspool.tile([128, NLOC], f32, tag="ps",
                                   name=f"ps_a1_{b}_{m}") for m in ms]
                for mi, m in enumerate(ms):
                    nc.tensor.matmul(pss[mi][:],
                                     mw1ft[:, 0:NF81,
                                           m * 128:(m + 1) * 128],
                                     a0f[:],
                                     start=True, stop=False, perf_mode=DR)
                for k in range(NF81, KT):
                    for mi, m in enumerate(ms):
                        nc.tensor.matmul(pss[mi][:],
                                         mw1t[:, k - NF81,
                                              m * 128:(m + 1) * 128],
                                         a0[k][:],
                                         start=False, stop=(k == KT - 1))
                for mi, m in enumerate(ms):
                    if m < NF8:
                        at = a1f[m // 2][:, m % 2, :]
                    else:
                        ah = apool.tile([128, NLOC], f16, tag=f"a1h_{m}",
                                        name=f"a1h_{b}_{m}")
                        a1h[m - NF8] = ah
                        at = ah[:]
                    drain(at, pss[mi][:], KT + m, dve=(m % 2 == 1))

            if b == 0:
                # cold-window filler: mask 1's L0 weights are already on
                # chip while mask 0's L1/L2 weights still stream -- run
                # L0(1) here instead of letting the PE idle (also keeps the
                # HAM activity window filled)
                a0_cache[1] = layer0(1, fetched[1][0])
            elif b == 1:
                # L0(1) already ran; put the pipelined layer-3 here instead
                layer3_pairs(*prev, range(KT // 2))

            # ---- layer 2: fp8 DoubleRow pairs (k 0-3) then fp16 (k 4-7)
            a2 = [None] * KT
            for half in range(2):
                ms = range(half * 4, half * 4 + 4)
                pss = [pspool.tile([128, NLOC], f32, tag="ps",
                                   name=f"ps_a2_{b}_{m}") for m in ms]
                for t in range(NF8 // 2):
                    for mi, m in enumerate(ms):
                        nc.tensor.matmul(pss[mi][:],
                                         mw2ft[:, 2 * t:2 * t + 2,
                                               m * 128:(m + 1) * 128],
                                         a1f[t][:],
                                         start=(t == 0), stop=False,
                                         perf_mode=DR)
                for k in range(KT - NF8):
                    for mi, m in enumerate(ms):
                        nc.tensor.matmul(pss[mi][:],
                                         mw2ht[:, k, m * 128:(m + 1) * 128],
                                         a1h[k][:],
                                         start=False, stop=(k == KT - NF8 - 1))
                for mi, m in enumerate(ms):
                    at = apool.tile([128, NLOC], f16, tag=f"a2_{m}",
                                    name=f"a2_{b}_{m}")
                    if b == B - 1:
                        drain_split(at[:], pss[mi][:], 2 * KT + m)
                    else:
                        drain(at[:], pss[mi][:], 2 * KT + m, dve=(m % 2 == 1))
                    a2[m] = at
                if b == B - 1:
                    # last mask: no next L0 to pipeline behind -- issue the
                    # final ps3 accumulation as soon as each half drains
                    layer3_pairs(b, mw3t, a2, [2 * half, 2 * half + 1])

            prev = (b, mw3t, a2) if b < B - 1 else None

        # ---- finalize: sum the two col-group halves of ps3 with a fp16
        # selector matmul (halves are ~1e2 scale; fp16 rounding adds ~3e-4
        # rel err), apply mean+bias, DMA out transposed (host transposes)
        s3 = finp.tile([128, NLOC], f16, tag="s3")
        nc.scalar.copy(s3[:], ps3[:])
        psf = pspool.tile([D_OUT, NLOC], f32, tag="ps", name="psf")
        nc.tensor.matmul(psf[:], selt[:], s3[:], start=True, stop=True)
        outt = finp.tile([D_OUT, NLOC], f32, tag="outt")
        nc.scalar.activation(outt[:], psf[:], iden, bias=b3t[:, 0:1],
                             scale=1.0 / (SW * SA * B))
        nc.sync.dma_start(out_d[:], outt[:])

    nc.compile()
    return nc


def _prep_shared(W0, W1, W2, W3, b0, b1, b2, b3,
                 mask0, mask1, mask2, mask3):
    def kfold(a, out_w):
        # [1024, out] -> [8, 128, out] -> [128, 8, out]
        return np.ascontiguousarray(
            a.reshape(KT, 128, out_w).transpose(1, 0, 2))

    def mfold(m, out_w):
        # [B, 1024, out] -> [B, 128, 8, out]
        return np.ascontiguousarray(
            m.reshape(B, KT, 128, out_w).transpose(0, 2, 1, 3))

    # fp16-rounded weights, masked on host; scales folded in:
    #   mw1 *= SA  (psum1 = SA*z1 so fp8 a1 needs no drain scale)
    #   mw2 *= SW  (both fp8 and fp16 parts; psum2 = SA*SW*z2)
    # biases: b0, SA*b1, SA*SW*b2 -> every drain is max(psum+bias, 0)
    wt0 = np.asarray(W0.T, F16).astype(np.float32)
    wt1 = np.asarray(W1.T, F16).astype(np.float32)
    wt2 = np.asarray(W2.T, F16).astype(np.float32) * SW
    wt3 = np.asarray(W3.T, F16).astype(np.float32)

    # a0 m-tiles 0..NF81-1 are stored as SA0*a0 in fp8: fold SA0 into
    # mw0's first 256 output columns (and their biases)
    mw0 = (wt0[None] * mask0)
    mw0[:, :, :NF81 * 128] *= SA0
    mw0 = mw0.astype(F16)                                       # [B, 96, H]
    mw1a = mfold(wt1[None] * mask1, H)                          # fp32
    mw1f = (mw1a[:, :, :NF81, :] * SW1).astype(E4)
    mw1 = (mw1a[:, :, NF81:, :] * SA).astype(F16)               # [B,128,6,H]
    mw2 = mfold(wt2[None] * mask2, H)                           # fp32
    mw2f = mw2[:, :, :NF8, :].astype(E4)
    mw2h = mw2[:, :, NF8:, :].astype(F16)
    mw3 = mfold(wt3[None] * mask3, D_OUT).astype(F16)           # [B,128,8,64]

    def brs(v):
        return np.ascontiguousarray(v.reshape(KT, 128).T).astype(np.float32)

    b0s = brs(b0)
    b0s[:, :NF81] *= SA0
    biasp = np.concatenate([b0s, brs(SA * b1), brs(SA * SW * b2)],
                           axis=1)                               # [128, 24]
    sel = np.zeros((128, D_OUT), F16)
    sel[np.arange(D_OUT), np.arange(D_OUT)] = 1.0
    sel[np.arange(D_OUT) + D_OUT, np.arange(D_OUT)] = 1.0

    return dict(
        mw0=mw0, mw1f=mw1f, mw1=mw1, mw2f=mw2f, mw2h=mw2h, mw3=mw3,
        biasp=biasp, sel=sel,
        b3r=np.ascontiguousarray(b3.reshape(D_OUT, 1)).astype(np.float32),
    )


def kernel(xy, W0, b0, W1, b1, W2, b2, W3, b3,
           mask0, mask1, mask2, mask3):
    global LAST_RESULT
    xy = np.asarray(xy, np.float32)
    args = [np.asarray(a, np.float32) for a in
            (W0, W1, W2, W3, b0, b1, b2, b3)]
    masks = [np.asarray(m, np.float32) for m in (mask0, mask1, mask2, mask3)]

    if "nc" not in _CACHE:
        _CACHE["nc"] = _build_graph()
    nc = _CACHE["nc"]

    shared = _prep_shared(*args, *masks)
    xyT = np.ascontiguousarray(xy.T).astype(F16)   # [96, 4096]
    in_maps = []
    for core in range(NCORES):
        im = dict(shared)
        im["xyT"] = np.ascontiguousarray(
            xyT[:, core * NLOC:(core + 1) * NLOC])
        in_maps.append(im)

    if TRACE:
        _ensure_ntff_hook()
    res = run_bass_kernel_spmd(
        nc, in_maps, core_ids=list(range(NCORES)),
        trace=TRACE)
    LAST_RESULT = res
    return np.concatenate(
        [np.asarray(res.results[i]["out"], np.float32).T
         for i in range(NCORES)], axis=0)


# revision 36
# speedup vs baseline: 1.0015x; 1.0015x over previous
"""CMADE ensemble kernel for 8 TRN2 NeuronCores.

Problem: B=16 binary-masked 4-layer MLPs (96 -> 1024 -> 1024 -> 1024 -> 64)
over the same N=4096 batch; output = mean over the 16 masks.

Strategy: data-parallel over the batch N -- each core takes 512 rows and runs
all 16 masked MLPs on them, accumulating the final-layer outputs of all 16
masks into a single PSUM tile; no inter-core collective is needed.

v2 design (vs the mask-streaming baseline):
- Masked weights (W.T * M) are precomputed on the HOST and streamed to the
  device as ready-to-matmul fp16/fp8 tiles. This removes the entire
  DVE mask-multiply pipeline (340us of vector-engine work) and the
  DMA->DVE->PE dependency chain that stalled the PE at startup and between
  early masks. The device is pure weight-streaming + matmul + drains.
- Non-fp8 data is fp16 (not bf16): same PE speed, 8x less quantization
  error (rel err 3.9e-4 vs 3.3e-3), buying error budget for fp8.
- Layer 2's first 4 k-slices (half of L2 = 25% of the mid-layer MACs) run
  as fp8e4 DoubleRow pair-matmuls (2 fp8 MACs/cycle): weights scaled x64,
  activations x32, both quantized e4m3; psum stays exact fp32. Simulated
  end-to-end rel err 1.63e-2 (gate 2e-2). The scales are folded into host
  weight/bias prep so every drain is a plain max(psum+bias,0).
- Weight streams ride 3 independent DMA queues (sync-HWDGE: mw1,
  scalar-HWDGE: mw2, gpsimd-SWDGE: mw0/mw3), one coarse DMA per
  mask-layer (per-DMA descriptor-gen is ~1.2us, so few big DMAs beat many
  k-sliced ones); mask 0/1 are chunked so the first consumers start early.
- Finalize: the two L3 column-group psum halves are summed with a tiny
  fp32 selector matmul and the [64, 512] result is DMAed out transposed;
  the host does the final [512, 64] transpose (it concatenates anyway).
"""

import numpy as np
import ml_dtypes

from concourse import bacc
import concourse.bass as bass
import concourse.mybir as mybir
import concourse.tile as tile
from concourse.bass_utils import run_bass_kernel_spmd

F16 = np.float16
E4 = ml_dtypes.float8_e4m3

N = 4096
B = 16
NCORES = 8
NLOC = N // NCORES           # 512 batch rows per core
D_IN = 96
H = 1024
D_OUT = 64
KT = H // 128                # 8 k-tiles for the 1024-wide dims
NF8 = 4                      # leading k-slices of L2 in fp8 (DoubleRow pairs)
NF81 = 2                     # leading k-slices of L1 in fp8 (DoubleRow pairs)
SW = 64.0                    # L2 fp8 weight scale (folded into host weights)
SA = 32.0                    # a1 storage scale (folded into L1 wts/bias)
SA0 = 8.0                    # a0 fp8 storage scale (folded into mw0 cols 0:256)
SW1 = 4.0                    # L1 fp8 weight scale (SA0*SW1 == SA)

TRACE = False
LAST_RESULT = None

_CACHE = {}


def _ensure_ntff_hook():
    """The agent image's antenv lacks axon_hooks; reconstruct the NTFF
    profile hook from trn_agent_boot so trace=True yields exec_time_ns."""
    import sys as _sys
    import types
    try:
        from antenv import axon_hooks  # noqa: F401
        return
    except ImportError:
        pass
    import antenv
    import concourse.bass_utils as _bu
    _bu.upload_artifacts = lambda tmpdir: tmpdir  # zero-egress container
    holder = {}
    mod = types.ModuleType("antenv.axon_hooks")
    mod.set_axon_ntff_profile_hook = lambda h: holder.__setitem__("h", h)
    mod.get_axon_ntff_profile_hook = lambda: holder.get("h")
    _sys.modules["antenv.axon_hooks"] = mod
    antenv.axon_hooks = mod
    from trn_agent_boot.trn_boot import _ntff_profile_via_ctypes
    mod.set_axon_ntff_profile_hook(
        _ntff_profile_via_ctypes("/opt/axon/libaxon_pjrt.so"))


def _build_graph():
    f32 = mybir.dt.float32
    f16 = mybir.dt.float16
    f8 = mybir.dt.float8e4
    nc = bacc.Bacc("TRN2", target_bir_lowering=False, debug=False,
                   num_devices=NCORES)

    # ---- I/O ----
    xyT_d = nc.dram_tensor("xyT", [D_IN, NLOC], f16, kind="ExternalInput")
    mw0_d = nc.dram_tensor("mw0", [B, D_IN, H], f16, kind="ExternalInput")
    mw1f_d = nc.dram_tensor("mw1f", [B, 128, NF81, H], f8,
                            kind="ExternalInput")
    mw1_d = nc.dram_tensor("mw1", [B, 128, KT - NF81, H], f16,
                           kind="ExternalInput")
    mw2f_d = nc.dram_tensor("mw2f", [B, 128, NF8, H], f8, kind="ExternalInput")
    mw2h_d = nc.dram_tensor("mw2h", [B, 128, KT - NF8, H], f16,
                            kind="ExternalInput")
    mw3_d = nc.dram_tensor("mw3", [B, 128, KT, D_OUT], f16,
                           kind="ExternalInput")
    bias_d = nc.dram_tensor("biasp", [128, 3 * KT], f32, kind="ExternalInput")
    b3_d = nc.dram_tensor("b3r", [D_OUT, 1], f32, kind="ExternalInput")
    sel_d = nc.dram_tensor("sel", [128, D_OUT], f16, kind="ExternalInput")
    out_d = nc.dram_tensor("out", [D_OUT, NLOC], f32, kind="ExternalOutput")

    relu = mybir.ActivationFunctionType.Relu
    iden = mybir.ActivationFunctionType.Identity
    add_op = mybir.AluOpType.add
    max_op = mybir.AluOpType.max
    DR = mybir.MatmulPerfMode.DoubleRow

    from contextlib import ExitStack
    with tile.TileContext(nc) as tc, ExitStack() as ctx:
        const = ctx.enter_context(tc.tile_pool(name="const", bufs=1))
        mw0p = ctx.enter_context(tc.tile_pool(name="mw0", bufs=2))
        mw1fp = ctx.enter_context(tc.tile_pool(name="mw1f", bufs=2))
        mw1p = ctx.enter_context(tc.tile_pool(name="mw1", bufs=2))
        mw2fp = ctx.enter_context(tc.tile_pool(name="mw2f", bufs=2))
        mw2hp = ctx.enter_context(tc.tile_pool(name="mw2h", bufs=2))
        mw3p = ctx.enter_context(tc.tile_pool(name="mw3", bufs=2))
        apool = ctx.enter_context(tc.tile_pool(name="act", bufs=2))
        pspool = ctx.enter_context(tc.tile_pool(name="ps", bufs=7, space="PSUM"))
        ps3pool = ctx.enter_context(tc.tile_pool(name="ps3", bufs=1, space="PSUM"))
        finp = ctx.enter_context(tc.tile_pool(name="fin", bufs=2))

        # ---- startup-critical loads: mw0[0] rides the scalar HWDGE queue
        # (it ramps to full rate immediately); xyT + biases lead the sync
        # queue ahead of the mw1 chunks
        xyT = const.tile([D_IN, NLOC], f16, tag="xyT")
        nc.sync.dma_start(xyT[:], xyT_d[:])
        bt = const.tile([128, 3 * KT], f32, tag="bt")
        nc.sync.dma_start(bt[:], bias_d[:])

        # ---- per-mask weight streams; mask 0/1 chunked so early k-slices
        # post their completion semaphores before the whole layer arrives
        def fetch(b):
            mw0t = mw0p.tile([D_IN, H], f16, tag="mw0", bufs=3,
                             name=f"mw0_{b}")
            if b == 0:
                nc.scalar.dma_start(mw0t[:], mw0_d[b])
            else:
                nc.gpsimd.dma_start(mw0t[:], mw0_d[b])
            mw1ft = mw1fp.tile([128, NF81, H], f8, tag="mw1f", bufs=3,
                               name=f"mw1f_{b}")
            mw1t = mw1p.tile([128, KT - NF81, H], f16, tag="mw1", bufs=3,
                             name=f"mw1_{b}")
            if b == 0:
                # mask 0's first chunks on sync (behind xyT+bias), the rest
                # on the otherwise-idle scalar queue -- its few issues
                # retire before the first drains need the scalar engine
                nc.sync.dma_start(mw1ft[:], mw1f_d[b])
                nc.sync.dma_start(mw1t[:, 0:2, :], mw1_d[b][:, 0:2, :])
                nc.scalar.dma_start(mw1t[:, 2:4, :], mw1_d[b][:, 2:4, :])
                nc.scalar.dma_start(mw1t[:, 4:6, :], mw1_d[b][:, 4:6, :])
            elif b == 1:
                # split mask 1 across the scalar and sync queues so its L1
                # weights land before the cold-phase k-loop reaches them
                nc.scalar.dma_start(mw1ft[:], mw1f_d[b])
                nc.scalar.dma_start(mw1t[:, 0:2, :], mw1_d[b][:, 0:2, :])
                nc.sync.dma_start(mw1t[:, 2:4, :], mw1_d[b][:, 2:4, :])
                nc.sync.dma_start(mw1t[:, 4:6, :], mw1_d[b][:, 4:6, :])
            else:
                nc.sync.dma_start(mw1ft[:], mw1f_d[b])
                nc.sync.dma_start(mw1t[:], mw1_d[b])
            # mw2/mw3 ride the gpsimd SWDGE queue: DMA-issue instructions can
            # block on semaphore-reuse waits, and gpsimd has no compute
            # duties to stall (the scalar engine must stay free for drains)
            mw2ft = mw2fp.tile([128, NF8, H], f8, tag="mw2f", bufs=3,
                               name=f"mw2f_{b}")
            mw2ht = mw2hp.tile([128, KT - NF8, H], f16, tag="mw2h", bufs=3,
                               name=f"mw2h_{b}")
            if b < 2:
                nc.gpsimd.dma_start(mw2ft[:, 0:2, :], mw2f_d[b][:, 0:2, :])
                nc.gpsimd.dma_start(mw2ht[:, 0:2, :], mw2h_d[b][:, 0:2, :])
                nc.gpsimd.dma_start(mw2ft[:, 2:4, :], mw2f_d[b][:, 2:4, :])
                nc.gpsimd.dma_start(mw2ht[:, 2:4, :], mw2h_d[b][:, 2:4, :])
            else:
                nc.gpsimd.dma_start(mw2ft[:], mw2f_d[b])
                nc.gpsimd.dma_start(mw2ht[:], mw2h_d[b])
            mw3t = mw3p.tile([128, KT, D_OUT], f16, tag="mw3", bufs=3,
                             name=f"mw3_{b}")
            nc.gpsimd.dma_start(mw3t[:], mw3_d[b])
            return mw0t, mw1ft, mw1t, mw2ft, mw2ht, mw3t



        # psum -> sbuf drain: all scales are folded into weights/biases, so
        # every drain is max(psum + bias, 0), alternating Scalar/Vector
        def drain(at, ps, col, dve):
            if dve:
                nc.vector.tensor_scalar(at, ps, bt[:, col:col + 1], 0.0,
                                        add_op, max_op)
            else:
                nc.scalar.activation(at, ps, relu, bias=bt[:, col:col + 1])

        # latency-critical drains (a0 feeds L1's k-loop almost immediately):
        # split each tile across both engines by column half -- engine time
        # scales with the free dim, so halving columns halves the latency
        def drain_split(at, ps, col):
            h = NLOC // 2
            nc.scalar.activation(at[:, 0:h], ps[:, 0:h], relu,
                                 bias=bt[:, col:col + 1])
            nc.vector.tensor_scalar(at[:, h:NLOC], ps[:, h:NLOC],
                                    bt[:, col:col + 1], 0.0,
                                    add_op, max_op)

        ps3 = ps3pool.tile([128, NLOC], f32, tag="ps3")

        def layer3_pairs(b, mw3t, a2, kps):
            # M=64 fills half the PE columns; pair k-tiles into concurrent
            # col-group matmuls writing disjoint psum partition halves
            for kp in kps:
                k0, k1 = 2 * kp, 2 * kp + 1
                st = (b == 0 and kp == 0)
                sp = (b == B - 1 and kp == KT // 2 - 1)
                nc.tensor.matmul(ps3[0:D_OUT, :], mw3t[:, k0, :], a2[k0][:],
                                 start=st, stop=sp, tile_position=(0, 0))
                nc.tensor.matmul(ps3[D_OUT:128, :], mw3t[:, k1, :], a2[k1][:],
                                 start=st, stop=sp, tile_position=(0, 64))

        fetched = {0: fetch(0)}
        b3t = const.tile([D_OUT, 1], f32, tag="b3t")
        nc.gpsimd.dma_start(b3t[:], b3_d[:])
        selt = const.tile([128, D_OUT], f16, tag="selt")
        nc.gpsimd.dma_start(selt[:], sel_d[:])
        fetched[1] = fetch(1)

        prev = None  # (b, mw3t, a2) pending layer-3
        for b in range(B):
            if b + 2 < B:
                fetched[b + 2] = fetch(b + 2)
            mw0t, mw1ft, mw1t, mw2ft, mw2ht, mw3t = fetched.pop(b)

            # ---- layer 0: [96] -> [1024]; m-tiles 0..1 drain to the fp8
            # pair tile feeding L1's DoubleRow slices (psum is 8*z0 there --
            # the x8 is folded into mw0's first 256 columns)
            a0f = apool.tile([128, 2, NLOC], f8, tag="a0f",
                             name=f"a0f_{b}")
            a0 = [None] * KT
            for m in range(KT):
                ps = pspool.tile([128, NLOC], f32, tag="ps",
                                 name=f"ps_a0_{b}_{m}")
                nc.tensor.matmul(ps[:], mw0t[:, m * 128:(m + 1) * 128],
                                 xyT[:], start=True, stop=True)
                if m < NF81:
                    drain_split(a0f[:, m, :], ps[:], m)
                else:
                    at = apool.tile([128, NLOC], f16, tag=f"a0_{m}",
                                    name=f"a0_{b}_{m}")
                    drain_split(at[:], ps[:], m)
                    a0[m] = at

            if prev is not None:
                layer3_pairs(*prev, range(KT // 2))

            # ---- layer 1: k-outer over m-halves; m 0-3 drain to fp8 pair
            # tiles (L2's DoubleRow inputs), m 4-7 to fp16
            a1f = [apool.tile([128, 2, NLOC], f8, tag=f"a1f_{p}",
                              name=f"a1f_{b}_{p}") for p in range(NF8 // 2)]
            a1h = [None] * (KT - NF8)
            for half in range(2):
                ms = range(half * 4, half * 4 + 4)
                pss = [pspool.tile([128, NLOC], f32, tag="ps",
                                   name=f"ps_a1_{b}_{m}") for m in ms]
                for mi, m in enumerate(ms):
                    nc.tensor.matmul(pss[mi][:],
                                     mw1ft[:, 0:NF81,
                                           m * 128:(m + 1) * 128],
                                     a0f[:],
                                     start=True, stop=False, perf_mode=DR)
                for k in range(NF81, KT):
                    for mi, m in enumerate(ms):
                        nc.tensor.matmul(pss[mi][:],
                                         mw1t[:, k - NF81,
                                              m * 128:(m + 1) * 128],
                                         a0[k][:],
                                         start=False, stop=(k == KT - 1))
                for mi, m in enumerate(ms):
                    if m < NF8:
                        at = a1f[m // 2][:, m % 2, :]
                    else:
                        ah = apool.tile([128, NLOC], f16, tag=f"a1h_{m}",
                                        name=f"a1h_{b}_{m}")
                        a1h[m - NF8] = ah
                        at = ah[:]
                    drain(at, pss[mi][:], KT + m, dve=(m % 2 == 1))

            # ---- layer 2: fp8 DoubleRow pairs (k 0-3) then fp16 (k 4-7)
            a2 = [None] * KT
            for half in range(2):
                ms = range(half * 4, half * 4 + 4)
                pss = [pspool.tile([128, NLOC], f32, tag="ps",
                                   name=f"ps_a2_{b}_{m}") for m in ms]
                for t in range(NF8 // 2):
                    for mi, m in enumerate(ms):
                        nc.tensor.matmul(pss[mi][:],
                                         mw2ft[:, 2 * t:2 * t + 2,
                                               m * 128:(m + 1) * 128],
                                         a1f[t][:],
                                         start=(t == 0), stop=False,
                                         perf_mode=DR)
                for k in range(KT - NF8):
                    for mi, m in enumerate(ms):
                        nc.tensor.matmul(pss[mi][:],
                                         mw2ht[:, k, m * 128:(m + 1) * 128],
                                         a1h[k][:],
                                         start=False, stop=(k == KT - NF8 - 1))
                for mi, m in enumerate(ms):
                    at = apool.tile([128, NLOC], f16, tag=f"a2_{m}",
                                    name=f"a2_{b}_{m}")
                    if b == B - 1:
                        drain_split(at[:], pss[mi][:], 2 * KT + m)
                    else:
                        drain(at[:], pss[mi][:], 2 * KT + m, dve=(m % 2 == 1))
                    a2[m] = at
                if b == B - 1:
                    # last mask: no next L0 to pipeline behind -- issue the
                    # final ps3 accumulation as soon as each half drains
                    layer3_pairs(b, mw3t, a2, [2 * half, 2 * half + 1])

            prev = (b, mw3t, a2) if b < B - 1 else None

        # ---- finalize: sum the two col-group halves of ps3 with a fp16
        # selector matmul (halves are ~1e2 scale; fp16 rounding adds ~3e-4
        # rel err), apply mean+bias, DMA out transposed (host transposes)
        s3 = finp.tile([128, NLOC], f16, tag="s3")
        nc.scalar.copy(s3[:], ps3[:])
        psf = pspool.tile([D_OUT, NLOC], f32, tag="ps", name="psf")
        nc.tensor.matmul(psf[:], selt[:], s3[:], start=True, stop=True)
        outt = finp.tile([D_OUT, NLOC], f32, tag="outt")
        nc.scalar.activation(outt[:], psf[:], iden, bias=b3t[:, 0:1],
                             scale=1.0 / (SW * SA * B))
        nc.sync.dma_start(out_d[:], outt[:])

    nc.compile()
    return nc


def _prep_shared(W0, W1, W2, W3, b0, b1, b2, b3,
                 mask0, mask1, mask2, mask3):
    def kfold(a, out_w):
        # [1024, out] -> [8, 128, out] -> [128, 8, out]
        return np.ascontiguousarray(
            a.reshape(KT, 128, out_w).transpose(1, 0, 2))

    def mfold(m, out_w):
        # [B, 1024, out] -> [B, 128, 8, out]
        return np.ascontiguousarray(
            m.reshape(B, KT, 128, out_w).transpose(0, 2, 1, 3))

    # fp16-rounded weights, masked on host; scales folded in:
    #   mw1 *= SA  (psum1 = SA*z1 so fp8 a1 needs no drain scale)
    #   mw2 *= SW  (both fp8 and fp16 parts; psum2 = SA*SW*z2)
    # biases: b0, SA*b1, SA*SW*b2 -> every drain is max(psum+bias, 0)
    wt0 = np.asarray(W0.T, F16).astype(np.float32)
    wt1 = np.asarray(W1.T, F16).astype(np.float32)
    wt2 = np.asarray(W2.T, F16).astype(np.float32) * SW
    wt3 = np.asarray(W3.T, F16).astype(np.float32)

    # a0 m-tiles 0..NF81-1 are stored as SA0*a0 in fp8: fold SA0 into
    # mw0's first 256 output columns (and their biases)
    mw0 = (wt0[None] * mask0)
    mw0[:, :, :NF81 * 128] *= SA0
    mw0 = mw0.astype(F16)                                       # [B, 96, H]
    mw1a = mfold(wt1[None] * mask1, H)                          # fp32
    mw1f = (mw1a[:, :, :NF81, :] * SW1).astype(E4)
    mw1 = (mw1a[:, :, NF81:, :] * SA).astype(F16)               # [B,128,6,H]
    mw2 = mfold(wt2[None] * mask2, H)                           # fp32
    mw2f = mw2[:, :, :NF8, :].astype(E4)
    mw2h = mw2[:, :, NF8:, :].astype(F16)
    mw3 = mfold(wt3[None] * mask3, D_OUT).astype(F16)           # [B,128,8,64]

    def brs(v):
        return np.ascontiguousarray(v.reshape(KT, 128).T).astype(np.float32)

    b0s = brs(b0)
    b0s[:, :NF81] *= SA0
    biasp = np.concatenate([b0s, brs(SA * b1), brs(SA * SW * b2)],
                           axis=1)                               # [128, 24]
    sel = np.zeros((128, D_OUT), F16)
    sel[np.arange(D_OUT), np.arange(D_OUT)] = 1.0
    sel[np.arange(D_OUT) + D_OUT, np.arange(D_OUT)] = 1.0

    return dict(
        mw0=mw0, mw1f=mw1f, mw1=mw1, mw2f=mw2f, mw2h=mw2h, mw3=mw3,
        biasp=biasp, sel=sel,
        b3r=np.ascontiguousarray(b3.reshape(D_OUT, 1)).astype(np.float32),
    )


def kernel(xy, W0, b0, W1, b1, W2, b2, W3, b3,
           mask0, mask1, mask2, mask3):
    global LAST_RESULT
    xy = np.asarray(xy, np.float32)
    args = [np.asarray(a, np.float32) for a in
            (W0, W1, W2, W3, b0, b1, b2, b3)]
    masks = [np.asarray(m, np.float32) for m in (mask0, mask1, mask2, mask3)]

    if "nc" not in _CACHE:
        _CACHE["nc"] = _build_graph()
    nc = _CACHE["nc"]

    shared = _prep_shared(*args, *masks)
    xyT = np.ascontiguousarray(xy.T).astype(F16)   # [96, 4096]
    in_maps = []
    for core in range(NCORES):
        im = dict(shared)
        im["xyT"] = np.ascontiguousarray(
            xyT[:, core * NLOC:(core + 1) * NLOC])
        in_maps.append(im)

    if TRACE:
        _ensure_ntff_hook()
    res = run_bass_kernel_spmd(
        nc, in_maps, core_ids=list(range(NCORES)),
        trace=TRACE)
    LAST_RESULT = res
    return np.concatenate(
        [np.asarray(res.results[i]["out"], np.float32).T
         for i in range(NCORES)], axis=0)


# revision 37
# speedup vs baseline: 1.0026x; 1.0012x over previous
"""CMADE ensemble kernel for 8 TRN2 NeuronCores.

Problem: B=16 binary-masked 4-layer MLPs (96 -> 1024 -> 1024 -> 1024 -> 64)
over the same N=4096 batch; output = mean over the 16 masks.

Strategy: data-parallel over the batch N -- each core takes 512 rows and runs
all 16 masked MLPs on them, accumulating the final-layer outputs of all 16
masks into a single PSUM tile; no inter-core collective is needed.

v2 design (vs the mask-streaming baseline):
- Masked weights (W.T * M) are precomputed on the HOST and streamed to the
  device as ready-to-matmul fp16/fp8 tiles. This removes the entire
  DVE mask-multiply pipeline (340us of vector-engine work) and the
  DMA->DVE->PE dependency chain that stalled the PE at startup and between
  early masks. The device is pure weight-streaming + matmul + drains.
- Non-fp8 data is fp16 (not bf16): same PE speed, 8x less quantization
  error (rel err 3.9e-4 vs 3.3e-3), buying error budget for fp8.
- Layer 2's first 4 k-slices (half of L2 = 25% of the mid-layer MACs) run
  as fp8e4 DoubleRow pair-matmuls (2 fp8 MACs/cycle): weights scaled x64,
  activations x32, both quantized e4m3; psum stays exact fp32. Simulated
  end-to-end rel err 1.63e-2 (gate 2e-2). The scales are folded into host
  weight/bias prep so every drain is a plain max(psum+bias,0).
- Weight streams ride 3 independent DMA queues (sync-HWDGE: mw1,
  scalar-HWDGE: mw2, gpsimd-SWDGE: mw0/mw3), one coarse DMA per
  mask-layer (per-DMA descriptor-gen is ~1.2us, so few big DMAs beat many
  k-sliced ones); mask 0/1 are chunked so the first consumers start early.
- Finalize: the two L3 column-group psum halves are summed with a tiny
  fp32 selector matmul and the [64, 512] result is DMAed out transposed;
  the host does the final [512, 64] transpose (it concatenates anyway).
"""

import numpy as np
import ml_dtypes

from concourse import bacc
import concourse.bass as bass
import concourse.mybir as mybir
import concourse.tile as tile
from concourse.bass_utils import run_bass_kernel_spmd

F16 = np.float16
E4 = ml_dtypes.float8_e4m3

N = 4096
B = 16
NCORES = 8
NLOC = N // NCORES           # 512 batch rows per core
D_IN = 96
H = 1024
D_OUT = 64
KT = H // 128                # 8 k-tiles for the 1024-wide dims
NF8 = 4                      # leading k-slices of L2 in fp8 (DoubleRow pairs)
NF81 = 2                     # leading k-slices of L1 in fp8 (DoubleRow pairs)
SW = 64.0                    # L2 fp8 weight scale (folded into host weights)
SA = 32.0                    # a1 storage scale (folded into L1 wts/bias)
SA0 = 8.0                    # a0 fp8 storage scale (folded into mw0 cols 0:256)
SW1 = 4.0                    # L1 fp8 weight scale (SA0*SW1 == SA)

TRACE = False
LAST_RESULT = None

_CACHE = {}


def _ensure_ntff_hook():
    """The agent image's antenv lacks axon_hooks; reconstruct the NTFF
    profile hook from trn_agent_boot so trace=True yields exec_time_ns."""
    import sys as _sys
    import types
    try:
        from antenv import axon_hooks  # noqa: F401
        return
    except ImportError:
        pass
    import antenv
    import concourse.bass_utils as _bu
    _bu.upload_artifacts = lambda tmpdir: tmpdir  # zero-egress container
    holder = {}
    mod = types.ModuleType("antenv.axon_hooks")
    mod.set_axon_ntff_profile_hook = lambda h: holder.__setitem__("h", h)
    mod.get_axon_ntff_profile_hook = lambda: holder.get("h")
    _sys.modules["antenv.axon_hooks"] = mod
    antenv.axon_hooks = mod
    from trn_agent_boot.trn_boot import _ntff_profile_via_ctypes
    mod.set_axon_ntff_profile_hook(
        _ntff_profile_via_ctypes("/opt/axon/libaxon_pjrt.so"))


def _build_graph():
    f32 = mybir.dt.float32
    f16 = mybir.dt.float16
    f8 = mybir.dt.float8e4
    nc = bacc.Bacc("TRN2", target_bir_lowering=False, debug=False,
                   num_devices=NCORES)

    # ---- I/O ----
    xyT_d = nc.dram_tensor("xyT", [D_IN, NLOC], f16, kind="ExternalInput")
    mw0_d = nc.dram_tensor("mw0", [B, D_IN, H], f16, kind="ExternalInput")
    mw1f_d = nc.dram_tensor("mw1f", [B, 128, NF81, H], f8,
                            kind="ExternalInput")
    mw1_d = nc.dram_tensor("mw1", [B, 128, KT - NF81, H], f16,
                           kind="ExternalInput")
    mw2f_d = nc.dram_tensor("mw2f", [B, 128, NF8, H], f8, kind="ExternalInput")
    mw2h_d = nc.dram_tensor("mw2h", [B, 128, KT - NF8, H], f16,
                            kind="ExternalInput")
    mw3_d = nc.dram_tensor("mw3", [B, 128, KT, D_OUT], f16,
                           kind="ExternalInput")
    bias_d = nc.dram_tensor("biasp", [128, 3 * KT], f32, kind="ExternalInput")
    b3_d = nc.dram_tensor("b3r", [D_OUT, 1], f32, kind="ExternalInput")
    sel_d = nc.dram_tensor("sel", [128, D_OUT], f16, kind="ExternalInput")
    out_d = nc.dram_tensor("out", [D_OUT, NLOC], f32, kind="ExternalOutput")

    relu = mybir.ActivationFunctionType.Relu
    iden = mybir.ActivationFunctionType.Identity
    add_op = mybir.AluOpType.add
    max_op = mybir.AluOpType.max
    DR = mybir.MatmulPerfMode.DoubleRow

    from contextlib import ExitStack
    with tile.TileContext(nc) as tc, ExitStack() as ctx:
        const = ctx.enter_context(tc.tile_pool(name="const", bufs=1))
        mw0p = ctx.enter_context(tc.tile_pool(name="mw0", bufs=2))
        mw1fp = ctx.enter_context(tc.tile_pool(name="mw1f", bufs=2))
        mw1p = ctx.enter_context(tc.tile_pool(name="mw1", bufs=2))
        mw2fp = ctx.enter_context(tc.tile_pool(name="mw2f", bufs=2))
        mw2hp = ctx.enter_context(tc.tile_pool(name="mw2h", bufs=2))
        mw3p = ctx.enter_context(tc.tile_pool(name="mw3", bufs=2))
        apool = ctx.enter_context(tc.tile_pool(name="act", bufs=2))
        pspool = ctx.enter_context(tc.tile_pool(name="ps", bufs=7, space="PSUM"))
        ps3pool = ctx.enter_context(tc.tile_pool(name="ps3", bufs=1, space="PSUM"))
        finp = ctx.enter_context(tc.tile_pool(name="fin", bufs=2))

        # ---- startup-critical loads: mw0[0] rides the scalar HWDGE queue
        # (it ramps to full rate immediately); xyT + biases lead the sync
        # queue ahead of the mw1 chunks
        xyT = const.tile([D_IN, NLOC], f16, tag="xyT")
        nc.sync.dma_start(xyT[:], xyT_d[:])
        bt = const.tile([128, 3 * KT], f32, tag="bt")
        nc.sync.dma_start(bt[:], bias_d[:])

        # ---- per-mask weight streams; mask 0/1 chunked so early k-slices
        # post their completion semaphores before the whole layer arrives
        def fetch(b):
            mw0t = mw0p.tile([D_IN, H], f16, tag="mw0", bufs=3,
                             name=f"mw0_{b}")
            if b == 0:
                nc.scalar.dma_start(mw0t[:], mw0_d[b])
            else:
                nc.gpsimd.dma_start(mw0t[:], mw0_d[b])
            mw1ft = mw1fp.tile([128, NF81, H], f8, tag="mw1f", bufs=3,
                               name=f"mw1f_{b}")
            mw1t = mw1p.tile([128, KT - NF81, H], f16, tag="mw1", bufs=3,
                             name=f"mw1_{b}")
            if b == 0:
                # earliest-needed L1 weights (DR pairs + k2/k3) ride the
                # fast-ramping scalar queue right behind mw0[0]; the sync
                # queue (slow early ramp, also carrying xyT+bias) gets the
                # later k4..k7 chunks. Same issue counts per engine, only
                # the payloads swap -- measured sem-post times: mw1f 20.1us
                # and c01 24.2us on sync vs L1(0) needing them at ~14/16us
                nc.scalar.dma_start(mw1ft[:], mw1f_d[b])
                nc.scalar.dma_start(mw1t[:, 0:2, :], mw1_d[b][:, 0:2, :])
                nc.sync.dma_start(mw1t[:, 2:4, :], mw1_d[b][:, 2:4, :])
                nc.sync.dma_start(mw1t[:, 4:6, :], mw1_d[b][:, 4:6, :])
            elif b == 1:
                # split mask 1 across the scalar and sync queues so its L1
                # weights land before the cold-phase k-loop reaches them
                nc.scalar.dma_start(mw1ft[:], mw1f_d[b])
                nc.scalar.dma_start(mw1t[:, 0:2, :], mw1_d[b][:, 0:2, :])
                nc.sync.dma_start(mw1t[:, 2:4, :], mw1_d[b][:, 2:4, :])
                nc.sync.dma_start(mw1t[:, 4:6, :], mw1_d[b][:, 4:6, :])
            else:
                nc.sync.dma_start(mw1ft[:], mw1f_d[b])
                nc.sync.dma_start(mw1t[:], mw1_d[b])
            # mw2/mw3 ride the gpsimd SWDGE queue: DMA-issue instructions can
            # block on semaphore-reuse waits, and gpsimd has no compute
            # duties to stall (the scalar engine must stay free for drains)
            mw2ft = mw2fp.tile([128, NF8, H], f8, tag="mw2f", bufs=3,
                               name=f"mw2f_{b}")
            mw2ht = mw2hp.tile([128, KT - NF8, H], f16, tag="mw2h", bufs=3,
                               name=f"mw2h_{b}")
            if b < 2:
                nc.gpsimd.dma_start(mw2ft[:, 0:2, :], mw2f_d[b][:, 0:2, :])
                nc.gpsimd.dma_start(mw2ht[:, 0:2, :], mw2h_d[b][:, 0:2, :])
                nc.gpsimd.dma_start(mw2ft[:, 2:4, :], mw2f_d[b][:, 2:4, :])
                nc.gpsimd.dma_start(mw2ht[:, 2:4, :], mw2h_d[b][:, 2:4, :])
            else:
                nc.gpsimd.dma_start(mw2ft[:], mw2f_d[b])
                nc.gpsimd.dma_start(mw2ht[:], mw2h_d[b])
            mw3t = mw3p.tile([128, KT, D_OUT], f16, tag="mw3", bufs=3,
                             name=f"mw3_{b}")
            nc.gpsimd.dma_start(mw3t[:], mw3_d[b])
            return mw0t, mw1ft, mw1t, mw2ft, mw2ht, mw3t



        # psum -> sbuf drain: all scales are folded into weights/biases, so
        # every drain is max(psum + bias, 0), alternating Scalar/Vector
        def drain(at, ps, col, dve):
            if dve:
                nc.vector.tensor_scalar(at, ps, bt[:, col:col + 1], 0.0,
                                        add_op, max_op)
            else:
                nc.scalar.activation(at, ps, relu, bias=bt[:, col:col + 1])

        # latency-critical drains (a0 feeds L1's k-loop almost immediately):
        # split each tile across both engines by column half -- engine time
        # scales with the free dim, so halving columns halves the latency
        def drain_split(at, ps, col):
            h = NLOC // 2
            nc.scalar.activation(at[:, 0:h], ps[:, 0:h], relu,
                                 bias=bt[:, col:col + 1])
            nc.vector.tensor_scalar(at[:, h:NLOC], ps[:, h:NLOC],
                                    bt[:, col:col + 1], 0.0,
                                    add_op, max_op)

        ps3 = ps3pool.tile([128, NLOC], f32, tag="ps3")

        def layer3_pairs(b, mw3t, a2, kps):
            # M=64 fills half the PE columns; pair k-tiles into concurrent
            # col-group matmuls writing disjoint psum partition halves
            for kp in kps:
                k0, k1 = 2 * kp, 2 * kp + 1
                st = (b == 0 and kp == 0)
                sp = (b == B - 1 and kp == KT // 2 - 1)
                nc.tensor.matmul(ps3[0:D_OUT, :], mw3t[:, k0, :], a2[k0][:],
                                 start=st, stop=sp, tile_position=(0, 0))
                nc.tensor.matmul(ps3[D_OUT:128, :], mw3t[:, k1, :], a2[k1][:],
                                 start=st, stop=sp, tile_position=(0, 64))

        fetched = {0: fetch(0)}
        b3t = const.tile([D_OUT, 1], f32, tag="b3t")
        nc.gpsimd.dma_start(b3t[:], b3_d[:])
        selt = const.tile([128, D_OUT], f16, tag="selt")
        nc.gpsimd.dma_start(selt[:], sel_d[:])
        fetched[1] = fetch(1)

        prev = None  # (b, mw3t, a2) pending layer-3
        for b in range(B):
            if b + 2 < B:
                fetched[b + 2] = fetch(b + 2)
            mw0t, mw1ft, mw1t, mw2ft, mw2ht, mw3t = fetched.pop(b)

            # ---- layer 0: [96] -> [1024]; m-tiles 0..1 drain to the fp8
            # pair tile feeding L1's DoubleRow slices (psum is 8*z0 there --
            # the x8 is folded into mw0's first 256 columns)
            a0f = apool.tile([128, 2, NLOC], f8, tag="a0f",
                             name=f"a0f_{b}")
            a0 = [None] * KT
            for m in range(KT):
                ps = pspool.tile([128, NLOC], f32, tag="ps",
                                 name=f"ps_a0_{b}_{m}")
                nc.tensor.matmul(ps[:], mw0t[:, m * 128:(m + 1) * 128],
                                 xyT[:], start=True, stop=True)
                if m < NF81:
                    drain_split(a0f[:, m, :], ps[:], m)
                else:
                    at = apool.tile([128, NLOC], f16, tag=f"a0_{m}",
                                    name=f"a0_{b}_{m}")
                    drain_split(at[:], ps[:], m)
                    a0[m] = at

            if prev is not None:
                layer3_pairs(*prev, range(KT // 2))

            # ---- layer 1: k-outer over m-halves; m 0-3 drain to fp8 pair
            # tiles (L2's DoubleRow inputs), m 4-7 to fp16
            a1f = [apool.tile([128, 2, NLOC], f8, tag=f"a1f_{p}",
                              name=f"a1f_{b}_{p}") for p in range(NF8 // 2)]
            a1h = [None] * (KT - NF8)
            for half in range(2):
                ms = range(half * 4, half * 4 + 4)
                pss = [pspool.tile([128, NLOC], f32, tag="ps",
                                   name=f"ps_a1_{b}_{m}") for m in ms]
                for mi, m in enumerate(ms):
                    nc.tensor.matmul(pss[mi][:],
                                     mw1ft[:, 0:NF81,
                                           m * 128:(m + 1) * 128],
                                     a0f[:],
                                     start=True, stop=False, perf_mode=DR)
                for k in range(NF81, KT):
                    for mi, m in enumerate(ms):
                        nc.tensor.matmul(pss[mi][:],
                                         mw1t[:, k - NF81,
                                              m * 128:(m + 1) * 128],
                                         a0[k][:],
                                         start=False, stop=(k == KT - 1))
                for mi, m in enumerate(ms):
                    if m < NF8:
                        at = a1f[m // 2][:, m % 2, :]
                    else:
                        ah = apool.tile([128, NLOC], f16, tag=f"a1h_{m}",
                                        name=f"a1h_{b}_{m}")
                        a1h[m - NF8] = ah
                        at = ah[:]
                    drain(at, pss[mi][:], KT + m, dve=(m % 2 == 1))

            # ---- layer 2: fp8 DoubleRow pairs (k 0-3) then fp16 (k 4-7)
            a2 = [None] * KT
            for half in range(2):
                ms = range(half * 4, half * 4 + 4)
                pss = [pspool.tile([128, NLOC], f32, tag="ps",
                                   name=f"ps_a2_{b}_{m}") for m in ms]
                for t in range(NF8 // 2):
                    for mi, m in enumerate(ms):
                        nc.tensor.matmul(pss[mi][:],
                                         mw2ft[:, 2 * t:2 * t + 2,
                                               m * 128:(m + 1) * 128],
                                         a1f[t][:],
                                         start=(t == 0), stop=False,
                                         perf_mode=DR)
                for k in range(KT - NF8):
                    for mi, m in enumerate(ms):
                        nc.tensor.matmul(pss[mi][:],
                                         mw2ht[:, k, m * 128:(m + 1) * 128],
                                         a1h[k][:],
                                         start=False, stop=(k == KT - NF8 - 1))
                for mi, m in enumerate(ms):
                    at = apool.tile([128, NLOC], f16, tag=f"a2_{m}",
                                    name=f"a2_{b}_{m}")
                    if b == B - 1:
                        drain_split(at[:], pss[mi][:], 2 * KT + m)
                    else:
                        drain(at[:], pss[mi][:], 2 * KT + m, dve=(m % 2 == 1))
                    a2[m] = at
                if b == B - 1:
                    # last mask: no next L0 to pipeline behind -- issue the
                    # final ps3 accumulation as soon as each half drains
                    layer3_pairs(b, mw3t, a2, [2 * half, 2 * half + 1])

            prev = (b, mw3t, a2) if b < B - 1 else None

        # ---- finalize: sum the two col-group halves of ps3 with a fp16
        # selector matmul (halves are ~1e2 scale; fp16 rounding adds ~3e-4
        # rel err), apply mean+bias, DMA out transposed (host transposes)
        s3 = finp.tile([128, NLOC], f16, tag="s3")
        nc.scalar.copy(s3[:], ps3[:])
        psf = pspool.tile([D_OUT, NLOC], f32, tag="ps", name="psf")
        nc.tensor.matmul(psf[:], selt[:], s3[:], start=True, stop=True)
        outt = finp.tile([D_OUT, NLOC], f32, tag="outt")
        nc.scalar.activation(outt[:], psf[:], iden, bias=b3t[:, 0:1],
                             scale=1.0 / (SW * SA * B))
        nc.sync.dma_start(out_d[:], outt[:])

    nc.compile()
    return nc


def _prep_shared(W0, W1, W2, W3, b0, b1, b2, b3,
                 mask0, mask1, mask2, mask3):
    def kfold(a, out_w):
        # [1024, out] -> [8, 128, out] -> [128, 8, out]
        return np.ascontiguousarray(
            a.reshape(KT, 128, out_w).transpose(1, 0, 2))

    def mfold(m, out_w):
        # [B, 1024, out] -> [B, 128, 8, out]
        return np.ascontiguousarray(
            m.reshape(B, KT, 128, out_w).transpose(0, 2, 1, 3))

    # fp16-rounded weights, masked on host; scales folded in:
    #   mw1 *= SA  (psum1 = SA*z1 so fp8 a1 needs no drain scale)
    #   mw2 *= SW  (both fp8 and fp16 parts; psum2 = SA*SW*z2)
    # biases: b0, SA*b1, SA*SW*b2 -> every drain is max(psum+bias, 0)
    wt0 = np.asarray(W0.T, F16).astype(np.float32)
    wt1 = np.asarray(W1.T, F16).astype(np.float32)
    wt2 = np.asarray(W2.T, F16).astype(np.float32) * SW
    wt3 = np.asarray(W3.T, F16).astype(np.float32)

    # a0 m-tiles 0..NF81-1 are stored as SA0*a0 in fp8: fold SA0 into
    # mw0's first 256 output columns (and their biases)
    mw0 = (wt0[None] * mask0)
    mw0[:, :, :NF81 * 128] *= SA0
    mw0 = mw0.astype(F16)                                       # [B, 96, H]
    mw1a = mfold(wt1[None] * mask1, H)                          # fp32
    mw1f = (mw1a[:, :, :NF81, :] * SW1).astype(E4)
    mw1 = (mw1a[:, :, NF81:, :] * SA).astype(F16)               # [B,128,6,H]
    mw2 = mfold(wt2[None] * mask2, H)                           # fp32
    mw2f = mw2[:, :, :NF8, :].astype(E4)
    mw2h = mw2[:, :, NF8:, :].astype(F16)
    mw3 = mfold(wt3[None] * mask3, D_OUT).astype(F16)           # [B,128,8,64]

    def brs(v):
        return np.ascontiguousarray(v.reshape(KT, 128).T).astype(np.float32)

    b0s = brs(b0)
    b0s[:, :NF81] *= SA0
    biasp = np.concatenate([b0s, brs(SA * b1), brs(SA * SW * b2)],
                           axis=1)                               # [128, 24]
    sel = np.zeros((128, D_OUT), F16)
    sel[np.arange(D_OUT), np.arange(D_OUT)] = 1.0
    sel[np.arange(D_OUT) + D_OUT, np.arange(D_OUT)] = 1.0

    return dict(
        mw0=mw0, mw1f=mw1f, mw1=mw1, mw2f=mw2f, mw2h=mw2h, mw3=mw3,
        biasp=biasp, sel=sel,
        b3r=np.ascontiguousarray(b3.reshape(D_OUT, 1)).astype(np.float32),
    )


def kernel(xy, W0, b0, W1, b1, W2, b2, W3, b3,
           mask0, mask1, mask2, mask3):
    global LAST_RESULT
    xy = np.asarray(xy, np.float32)
    args = [np.asarray(a, np.float32) for a in
            (W0, W1, W2, W3, b0, b1, b2, b3)]
    masks = [np.asarray(m, np.float32) for m in (mask0, mask1, mask2, mask3)]

    if "nc" not in _CACHE:
        _CACHE["nc"] = _build_graph()
    nc = _CACHE["nc"]

    shared = _prep_shared(*args, *masks)
    xyT = np.ascontiguousarray(xy.T).astype(F16)   # [96, 4096]
    in_maps = []
    for core in range(NCORES):
        im = dict(shared)
        im["xyT"] = np.ascontiguousarray(
            xyT[:, core * NLOC:(core + 1) * NLOC])
        in_maps.append(im)

    if TRACE:
        _ensure_ntff_hook()
    res = run_bass_kernel_spmd(
        nc, in_maps, core_ids=list(range(NCORES)),
        trace=TRACE)
    LAST_RESULT = res
    return np.concatenate(
        [np.asarray(res.results[i]["out"], np.float32).T
         for i in range(NCORES)], axis=0)
